# revision 1
# baseline (speedup 1.0000x reference)
"""AttentionBlock (adaptive GroupNorm + spatial self-attention + residual)
Trainium2 Bass/Tile kernel, data-parallel over batch across 8 NeuronCores.

Reference computation (B=16, C=256, H=W=32, 8 heads x 32 dk, 8 GN groups):
  params = silu([t_emb, cond_emb]) @ proj_w.T + proj_b       (B, 512)
  xn = GroupNorm(x) * (1+gamma) + beta                        (B, C, 1024)
  qkv = xn.T @ qkv_w.T + qkv_b ; attention over 1024 positions
  out = attn_out @ out_w.T + out_b ; y = out + x

Per-core layout strategy (2 images/core):
  - everything kept in [channel/partition, spatial/free] layout
  - scores computed transposed  S_T[t, s] = K^T Q  via 2-head row-tiled
    matmuls (K=32 contraction), softmax exp on ScalarE (PSUM->SBUF, bf16)
  - unnormalized attn@v + denominator via 4-head col-tiled matmuls (M=32),
    denominator replicated to all 32 lanes of each head's partition strip so
    a single DVE reciprocal + tensor-tensor multiply normalizes the output
  - out-projection consumes [head*dk, s] directly; residual added on DVE
"""

import numpy as np
import ml_dtypes

B, C, HH, WW = 16, 256, 32, 32
S = HH * WW              # 1024
NH, DK = 8, 32           # heads x head_dim
G = 8                    # groupnorm groups
T_DIM, COND_DIM = 512, 128
IN_DIM = T_DIM + COND_DIM
EPS = 1e-6
NCORES = 8
BPC = B // NCORES        # images per core

_CACHE = {}

bf16 = ml_dtypes.bfloat16


def _build():
    """Build + compile the per-core Bacc graph (BPC images per core)."""
    import concourse.bacc as bacc
    import concourse.mybir as mybir
    import concourse.tile as tile
    from concourse.bass import ts, ds

    f32 = mybir.dt.float32
    b16 = mybir.dt.bfloat16
    AF = mybir.ActivationFunctionType
    OP = mybir.AluOpType

    nc = bacc.Bacc("TRN2", target_bir_lowering=False, num_devices=NCORES)

    # ---------------- DRAM parameters (host-preprocessed layouts) ----------
    x_ext = nc.declare_dram_parameter("x", [BPC, 2, 128, S], f32, isOutput=False)
    silu_in = nc.declare_dram_parameter("silu_in", [128, 5, BPC], f32, isOutput=False)
    proj_wt = nc.declare_dram_parameter("proj_wt", [128, 5, 512], b16, isOutput=False)
    proj_b = nc.declare_dram_parameter("proj_b", [128, 4, 1], f32, isOutput=False)
    qkw_t = nc.declare_dram_parameter("qkw_t", [128, 2, 512], b16, isOutput=False)
    qk_b = nc.declare_dram_parameter("qk_b", [128, 4, 1], f32, isOutput=False)
    vw_t = nc.declare_dram_parameter("vw_t", [128, 2, 256], b16, isOutput=False)
    v_b = nc.declare_dram_parameter("v_b", [1, 256], b16, isOutput=False)
    outw_t = nc.declare_dram_parameter("outw_t", [128, 2, 256], b16, isOutput=False)
    out_b = nc.declare_dram_parameter("out_b", [1, 256], b16, isOutput=False)
    gnw_p = nc.declare_dram_parameter("gnw", [128, 2, 1], f32, isOutput=False)
    gnb_p = nc.declare_dram_parameter("gnb", [128, 2, 1], f32, isOutput=False)
    ind_g = nc.declare_dram_parameter("ind_g", [128, 2, 8], f32, isOutput=False)
    ind_t = nc.declare_dram_parameter("ind_t", [8, 2, 128], f32, isOutput=False)
    ones1 = nc.declare_dram_parameter("ones1", [1, 128], b16, isOutput=False)
    ones512 = nc.declare_dram_parameter("ones512", [1, 512], b16, isOutput=False)
    out_ext = nc.declare_dram_parameter("out", [BPC, 2, 128, S], f32, isOutput=True)

    with tile.TileContext(nc) as tc:
        with (
            tc.tile_pool(name="const", bufs=1) as const,
            tc.tile_pool(name="xpool", bufs=2 * BPC) as xpool,
            tc.tile_pool(name="xn", bufs=2 * BPC) as xnpool,
            tc.tile_pool(name="qk", bufs=4 * BPC) as qkpool,
            tc.tile_pool(name="vp", bufs=8 * BPC) as vpool,
            tc.tile_pool(name="pp", bufs=6) as ppool,
            tc.tile_pool(name="on", bufs=2 * BPC) as onpool,
            tc.tile_pool(name="sm", bufs=4) as sm,
            tc.tile_pool(name="yp", bufs=4) as ypool,
            tc.tile_pool(name="psb", bufs=2, space="PSUM") as psb,
            tc.tile_pool(name="pss", bufs=4, space="PSUM") as pss,
        ):
            # ------------- constant / weight loads -------------------------
            projw_sb = const.tile([128, 5, 512], b16)
            nc.sync.dma_start(projw_sb[:], proj_wt[:])
            qkw_sb = const.tile([128, 2, 512], b16)
            nc.sync.dma_start(qkw_sb[:], qkw_t[:])
            vw_sb = const.tile([128, 2, 256], b16)
            nc.sync.dma_start(vw_sb[:], vw_t[:])
            outw_sb = const.tile([128, 2, 256], b16)
            nc.sync.dma_start(outw_sb[:], outw_t[:])
            projb_sb = const.tile([128, 4, 1], f32)
            nc.sync.dma_start(projb_sb[:], proj_b[:])
            qkb_sb = const.tile([128, 4, 1], f32)
            nc.sync.dma_start(qkb_sb[:], qk_b[:])
            vb_sb = const.tile([1, 256], b16)
            nc.sync.dma_start(vb_sb[:], v_b[:])
            outb_sb = const.tile([1, 256], b16)
            nc.sync.dma_start(outb_sb[:], out_b[:])
            gnw_sb = const.tile([128, 2, 1], f32)
            nc.sync.dma_start(gnw_sb[:], gnw_p[:])
            gnb_sb = const.tile([128, 2, 1], f32)
            nc.sync.dma_start(gnb_sb[:], gnb_p[:])
            indg_sb = const.tile([128, 2, 8], f32)
            nc.sync.dma_start(indg_sb[:], ind_g[:])
            indt_sb = const.tile([8, 2, 128], f32)
            nc.sync.dma_start(indt_sb[:], ind_t[:])
            ones1_sb = const.tile([1, 128], b16)
            nc.sync.dma_start(ones1_sb[:], ones1[:])
            ones512_sb = const.tile([1, 512], b16)
            nc.sync.dma_start(ones512_sb[:], ones512[:])
            silu_sb = const.tile([128, 5, BPC], f32)
            nc.sync.dma_start(silu_sb[:], silu_in[:])
            onescol = const.tile([128, 64], b16)
            nc.vector.memset(onescol[:], 1.0)
            eps_sb = const.tile([8, 1], f32)
            nc.vector.memset(eps_sb[:], EPS)

            # ------------- adaLN: silu + projection (both images) ----------
            sige = sm.tile([128, 5, BPC], f32, tag="sm")
            nc.scalar.activation(sige[:], silu_sb[:], AF.Exp, scale=-1.0)
            nc.vector.tensor_scalar_add(sige[:], sige[:], 1.0)
            nc.vector.reciprocal(sige[:], sige[:])
            silu_bf = sm.tile([128, 5, BPC], b16, tag="sm2")
            nc.vector.tensor_tensor(silu_bf[:], silu_sb[:], sige[:], OP.mult)

            params_ps = pss.tile([128, 4 * BPC], f32, tag="ps_s")
            for mt in range(4):
                for kt in range(5):
                    nc.tensor.matmul(
                        params_ps[:, mt * BPC:(mt + 1) * BPC],
                        lhsT=projw_sb[:, kt, ts(mt, 128)],
                        rhs=silu_bf[:, kt, :],
                        start=(kt == 0),
                        stop=(kt == 4),
                    )
            params_sb = sm.tile([128, 4, BPC], f32, tag="sm3")
            for mt in range(4):
                nc.vector.tensor_scalar_add(
                    params_sb[:, mt, :],
                    params_ps[:, mt * BPC:(mt + 1) * BPC],
                    projb_sb[:, mt, :],
                )

            for b in range(BPC):
                # ------------- load x, GN statistics -----------------------
                x_sb = []
                for ct in range(2):
                    xt = xpool.tile([128, S], f32, tag="x")
                    nc.sync.dma_start(xt[:], x_ext[b, ct])
                    x_sb.append(xt)

                me2 = []
                for ct in range(2):
                    st6 = sm.tile([128, 2, 6], f32, tag="st6")
                    for half in range(2):
                        nc.vector.bn_stats(
                            st6[:, half, :], x_sb[ct][:, ts(half, 512)]
                        )
                    mv = sm.tile([128, 2], f32, tag="mv")
                    nc.vector.bn_aggr(mv[:], st6[:])
                    m2 = sm.tile([128, 2], f32, tag="m2")
                    nc.vector.tensor_tensor(
                        m2[:, 1:2], mv[:, 0:1], mv[:, 0:1], OP.mult
                    )
                    nc.vector.tensor_tensor(
                        m2[:, 1:2], m2[:, 1:2], mv[:, 1:2], OP.add
                    )
                    nc.vector.tensor_copy(m2[:, 0:1], mv[:, 0:1])
                    me2.append(m2)

                gst_ps = pss.tile([8, 2], f32, tag="ps_s")
                for ct in range(2):
                    nc.tensor.matmul(
                        gst_ps[:],
                        lhsT=indg_sb[:, ct, :],
                        rhs=me2[ct][:],
                        start=(ct == 0),
                        stop=(ct == 1),
                    )
                gst = sm.tile([8, 2], f32, tag="gst")
                nc.vector.tensor_copy(gst[:], gst_ps[:])
                gvar = sm.tile([8, 1], f32, tag="gvar")
                nc.vector.tensor_tensor(gvar[:], gst[:, 0:1], gst[:, 0:1], OP.mult)
                nc.vector.tensor_tensor(gvar[:], gst[:, 1:2], gvar[:], OP.subtract)
                # rstd = exp(-0.5*ln(var+eps)) — keeps everything in the
                # exp/ln activation-table set (no table switch)
                lnv = sm.tile([8, 1], f32, tag="lnv")
                nc.scalar.activation(lnv[:], gvar[:], AF.Ln, bias=eps_sb[:])
                mr = sm.tile([8, 2], f32, tag="mr")
                nc.scalar.activation(mr[:, 1:2], lnv[:], AF.Exp, scale=-0.5)
                nc.vector.tensor_copy(mr[:, 0:1], gst[:, 0:1])

                xn3 = []
                for ct in range(2):
                    pc_ps = pss.tile([128, 2], f32, tag="ps_s")
                    nc.tensor.matmul(
                        pc_ps[:], lhsT=indt_sb[:, ct, :], rhs=mr[:],
                        start=True, stop=True,
                    )
                    gp1 = sm.tile([128, 1], f32, tag="gp1")
                    nc.vector.tensor_scalar_add(
                        gp1[:], params_sb[:, ct, b:b + 1], 1.0
                    )
                    av = sm.tile([128, 1], f32, tag="av")
                    nc.vector.tensor_tensor(av[:], gnw_sb[:, ct, :], gp1[:], OP.mult)
                    nc.vector.tensor_tensor(av[:], pc_ps[:, 1:2], av[:], OP.mult)
                    bv = sm.tile([128, 1], f32, tag="bv")
                    nc.vector.tensor_tensor(bv[:], pc_ps[:, 0:1], av[:], OP.mult)
                    bv2 = sm.tile([128, 1], f32, tag="bv2")
                    nc.vector.tensor_tensor(
                        bv2[:], gnb_sb[:, ct, :], gp1[:], OP.mult
                    )
                    nc.vector.tensor_tensor(bv2[:], bv2[:], bv[:], OP.subtract)
                    nc.vector.tensor_tensor(
                        bv2[:], bv2[:], params_sb[:, 2 + ct, b:b + 1], OP.add
                    )
                    xt = xnpool.tile([128, S], b16, tag="xn")
                    nc.vector.tensor_scalar(
                        xt[:], x_sb[ct][:], av[:], bv2[:], OP.mult, OP.add
                    )
                    xn3.append(xt)

                # ------------- qkv projections -----------------------------
                qk_sb = []
                for mt in range(4):      # 0,1 = q tiles; 2,3 = k tiles
                    ps_qk = psb.tile([128, S], f32, tag="ps_b")
                    for sc in range(2):
                        for kt in range(2):
                            nc.tensor.matmul(
                                ps_qk[:, ts(sc, 512)],
                                lhsT=qkw_sb[:, kt, ts(mt, 128)],
                                rhs=xn3[kt][:, ts(sc, 512)],
                                start=(kt == 0),
                                stop=(kt == 1),
                            )
                    qs = qkpool.tile([128, S], b16, tag="qk")
                    nc.vector.tensor_scalar_add(qs[:], ps_qk[:], qkb_sb[:, mt, :])
                    qk_sb.append(qs)

                # v tiles laid out as fused attn@v lhsT: per head a [128, 64]
                # block [v_h | ones] (even local head) or [ones | v_h] (odd),
                # so one M=64 chain yields out_u AND the replicated softmax
                # denominator in a single PE stream.
                v_sb = []
                for st in range(8):
                    ps_v = pss.tile([128, 256], f32, tag="ps_s")
                    for kt in range(2):
                        nc.tensor.matmul(
                            ps_v[:],
                            lhsT=xn3[kt][:, ts(st, 128)],
                            rhs=vw_sb[:, kt, :],
                            start=(kt == 0),
                            stop=False,
                        )
                    nc.tensor.matmul(
                        ps_v[:], lhsT=ones1_sb[:], rhs=vb_sb[:],
                        start=False, stop=True,
                    )
                    vt = vpool.tile([128, 8, 64], b16, tag="v")
                    # even local heads (h%4 in {0,2}) -> v in cols 0:32
                    nc.vector.tensor_copy(
                        vt[:, 0:8:2, 0:32],
                        ps_v[:].rearrange("p (h d) -> p h d", d=32)[:, 0:8:2, :],
                    )
                    nc.vector.tensor_copy(
                        vt[:, 1:8:2, 32:64],
                        ps_v[:].rearrange("p (h d) -> p h d", d=32)[:, 1:8:2, :],
                    )
                    nc.vector.memset(vt[:, 0:8:2, 32:64], 1.0)
                    nc.vector.memset(vt[:, 1:8:2, 0:32], 1.0)
                    v_sb.append(vt)

                # ------------- attention, two 4-head groups ----------------
                outn = []
                for g4 in range(2):
                    p_sb = [
                        ppool.tile([128, 8, S], b16, tag="p", name=f"p{i}")
                        for i in range(4)
                    ]
                    for pair in range(2):      # local heads (2*pair, 2*pair+1)
                        for st in range(8):
                            ps_sc = [
                                psb.tile([128, S], f32, tag="ps_b", name=f"sc{i}")
                                for i in range(2)
                            ]
                            for i in range(2):
                                hl = 2 * pair + i          # local head 0..3
                                h = 4 * g4 + hl
                                r = hl * 32                # PE row strip
                                for sc in range(2):
                                    nc.tensor.matmul(
                                        ps_sc[i][:, ts(sc, 512)],
                                        lhsT=qk_sb[2 + h // 4][
                                            ds(r, 32), ts(st, 128)
                                        ],
                                        rhs=qk_sb[h // 4][ds(r, 32), ts(sc, 512)],
                                        tile_position=(r, 0),
                                        start=True,
                                        stop=True,
                                    )
                            # exp(scores) PSUM->SBUF, bf16 — ScalarE
                            for i in range(2):
                                hl = 2 * pair + i
                                nc.scalar.activation(
                                    p_sb[hl][:, st, :], ps_sc[i][:], AF.Exp
                                )
                    # Fused attn@v + denominator: per local head one M=64
                    # chain with lhsT [v|ones] (even hl) or [ones|v] (odd),
                    # into its own PSUM bank. u lands at partitions 32*hl,
                    # the replicated denominator at the adjacent strip.
                    # Chains (hl, hl+2) use disjoint PE col groups and are
                    # st-interleaved for col-tiling concurrency.
                    for sc in range(2):
                        fused = [
                            pss.tile([128, 512], f32, tag="ps_s", name=f"fu{hl}")
                            for hl in range(4)
                        ]
                        for pas in range(2):       # pass 0: hl 0,2; 1: hl 1,3
                            for st in range(8):
                                for j in range(2):
                                    hl = pas + 2 * j
                                    base = 64 * (hl // 2)
                                    nc.tensor.matmul(
                                        fused[hl][ds(base, 64), :],
                                        lhsT=v_sb[st][:, 4 * g4 + hl, :],
                                        rhs=p_sb[hl][:, st, ts(sc, 512)],
                                        tile_position=(0, base),
                                        start=(st == 0),
                                        stop=(st == 7),
                                    )
                        if sc == 0:
                            ot = onpool.tile([128, S], b16, tag="on")
                            outn.append(ot)
                        for hl in range(4):
                            u = 32 * hl
                            dn = 64 * (hl // 2) + 32 * (1 - hl % 2)
                            rden = sm.tile(
                                [128, 512], f32, tag="rden", name=f"rd{hl}"
                            )
                            nc.vector.reciprocal(
                                rden[ds(dn, 32), :], fused[hl][ds(dn, 32), :]
                            )
                            nc.sync.dma_start(
                                rden[ds(u, 32), :], rden[ds(dn, 32), :]
                            )
                            nc.vector.tensor_tensor(
                                outn[g4][ds(u, 32), ts(sc, 512)],
                                fused[hl][ds(u, 32), :],
                                rden[ds(u, 32), :],
                                OP.mult,
                            )

                # ------------- output projection + residual ----------------
                for ct in range(2):
                    for sc in range(2):
                        ps_y = pss.tile([128, 512], f32, tag="ps_s")
                        for ot in range(2):
                            nc.tensor.matmul(
                                ps_y[:],
                                lhsT=outw_sb[:, ot, ts(ct, 128)],
                                rhs=outn[ot][:, ts(sc, 512)],
                                start=(ot == 0),
                                stop=False,
                            )
                        nc.tensor.matmul(
                            ps_y[:],
                            lhsT=outb_sb[:, ts(ct, 128)],
                            rhs=ones512_sb[:],
                            start=False,
                            stop=True,
                        )
                        yt = ypool.tile([128, 512], f32, tag="y")
                        nc.vector.tensor_tensor(
                            yt[:], ps_y[:], x_sb[ct][:, ts(sc, 512)], OP.add
                        )
                        nc.sync.dma_start(out_ext[b, ct, :, ts(sc, 512)], yt[:])

    nc.compile()
    return nc


def _prep_consts(inputs):
    """Host-side preprocessing of weights into device layouts (shared by all
    cores). Pure layout/dtype work — the math runs on device."""
    qkv_w = np.asarray(inputs["qkv_w"], np.float32)
    qkv_b = np.asarray(inputs["qkv_b"], np.float32)
    proj_w = np.asarray(inputs["proj_w"], np.float32)
    proj_b = np.asarray(inputs["proj_b"], np.float32)
    out_w = np.asarray(inputs["out_w"], np.float32)
    out_b = np.asarray(inputs["out_b"], np.float32)
    scale = 1.0 / np.sqrt(DK)

    wqk = qkv_w[:512].copy()          # q then k rows
    bqk = qkv_b[:512].copy()
    wqk[:256] *= scale                # fold 1/sqrt(dk) into q
    bqk[:256] *= scale
    wv = qkv_w[512:]
    bv = qkv_b[512:]

    d = {}
    d["proj_wt"] = np.ascontiguousarray(
        proj_w.T.reshape(5, 128, 512).transpose(1, 0, 2)
    ).astype(bf16)
    d["proj_b"] = np.ascontiguousarray(
        proj_b.reshape(4, 128).T.reshape(128, 4, 1)
    )
    d["qkw_t"] = np.ascontiguousarray(
        wqk.T.reshape(2, 128, 512).transpose(1, 0, 2)
    ).astype(bf16)
    d["qk_b"] = np.ascontiguousarray(bqk.reshape(4, 128).T.reshape(128, 4, 1))
    d["vw_t"] = np.ascontiguousarray(
        wv.T.reshape(2, 128, 256).transpose(1, 0, 2)
    ).astype(bf16)
    d["v_b"] = bv.reshape(1, 256).astype(bf16)
    d["outw_t"] = np.ascontiguousarray(
        out_w.T.reshape(2, 128, 256).transpose(1, 0, 2)
    ).astype(bf16)
    d["out_b"] = out_b.reshape(1, 256).astype(bf16)
    d["gnw"] = np.ascontiguousarray(
        np.asarray(inputs["gn_weight"], np.float32).reshape(2, 128).T
    ).reshape(128, 2, 1)
    d["gnb"] = np.ascontiguousarray(
        np.asarray(inputs["gn_bias"], np.float32).reshape(2, 128).T
    ).reshape(128, 2, 1)

    ind_g = np.zeros((128, 2, 8), np.float32)
    ind_t = np.zeros((8, 2, 128), np.float32)
    for ct in range(2):
        for p in range(128):
            g = (ct * 128 + p) // 32
            ind_g[p, ct, g] = 1.0 / 32.0
            ind_t[g, ct, p] = 1.0
    d["ind_g"] = ind_g
    d["ind_t"] = ind_t
    d["ones1"] = np.ones((1, 128), bf16)
    d["ones512"] = np.ones((1, 512), bf16)
    return d


def make_in_maps(inputs):
    consts = _prep_consts(inputs)
    x = np.asarray(inputs["x"], np.float32).reshape(B, 2, 128, S)
    t_emb = np.asarray(inputs["t_emb"], np.float32)
    cond_emb = np.asarray(inputs["cond_emb"], np.float32)
    inp_all = np.concatenate([t_emb, cond_emb], axis=1)       # (B, 640)

    in_maps = []
    for c in range(NCORES):
        m = dict(consts)
        m["x"] = np.ascontiguousarray(x[c * BPC:(c + 1) * BPC])
        sl = inp_all[c * BPC:(c + 1) * BPC].T                 # (640, BPC)
        m["silu_in"] = np.ascontiguousarray(
            sl.reshape(5, 128, BPC).transpose(1, 0, 2)
        )
        in_maps.append(m)
    return in_maps


def run(inputs, trace=False):
    from concourse.bass_utils import run_bass_kernel_spmd

    if "nc" not in _CACHE:
        _CACHE["nc"] = _build()
    nc = _CACHE["nc"]
    in_maps = make_in_maps(inputs)
    res = run_bass_kernel_spmd(
        nc, in_maps, core_ids=list(range(NCORES)), trace=trace
    )
    outs = [
        res.results[c]["out"].reshape(BPC, 256, HH, WW) for c in range(NCORES)
    ]
    y = np.concatenate(outs, axis=0)
    return y, res.exec_time_ns


def kernel(**inputs):
    y, _ = run(inputs, trace=False)
    return y



# revision 20
# speedup vs baseline: 1.1673x; 1.1673x over previous
"""AttentionBlock (adaptive GroupNorm + spatial self-attention + residual)
Trainium2 Bass/Tile kernel, data-parallel over batch across 8 NeuronCores.

Reference computation (B=16, C=256, H=W=32, 8 heads x 32 dk, 8 GN groups):
  params = silu([t_emb, cond_emb]) @ proj_w.T + proj_b       (B, 512)
  xn = GroupNorm(x) * (1+gamma) + beta                        (B, C, 1024)
  qkv = xn.T @ qkv_w.T + qkv_b ; attention over 1024 positions
  out = attn_out @ out_w.T + out_b ; y = out + x

Per-core layout strategy (2 images/core), v2:
  - everything kept in [channel/partition, spatial/free] layout
  - scores computed transposed  S_T[t, s] = K^T Q  via 2-head row-tiled
    matmuls (K=32 contraction)
  - softmax exp is the #1 bottleneck (8 * 1024^2 exps/image); it is split
    across THREE engines: ScalarE (exact activation exp) + DVE + GPSIMD
    (Schraudolph bit-trick exp: bf16 bits = int16(x*128/ln2 + 16250.5),
    ~1.7% mean element error, cancels in softmax normalization)
  - attn@v as 4-head col-tiled M=32 chains into one PSUM bank; softmax
    denominators via a second 4-head col-tiled ones-matmul pass into a
    sibling bank with IDENTICAL row alignment, so ONE full-width DVE
    reciprocal + ONE tensor_tensor normalizes all 4 heads at once
  - v bias folded into out_b on the host (out_b' = out_b + out_w @ v_b)
  - out-projection consumes [head*dk, s] directly; residual added on GPSIMD
"""

import numpy as np
import ml_dtypes

B, C, HH, WW = 16, 256, 32, 32
S = HH * WW              # 1024
NH, DK = 8, 32           # heads x head_dim
G = 8                    # groupnorm groups
T_DIM, COND_DIM = 512, 128
IN_DIM = T_DIM + COND_DIM
EPS = 1e-6
NCORES = 8
BPC = B // NCORES        # images per core

# Schraudolph exp constants for bf16-bits-via-int16:
#   bits = int16(x * 128/ln2 + (127*128 - 5.5))
A_EXP = 128.0 / np.log(2.0)
B_EXP = 127.0 * 128.0 - 5.5

# exp engine rotation: S=ScalarE exact, D=DVE schraudolph, P=GPSIMD schraudolph
EXP_PATTERN = "SDSDSDSDSDSDS"

_CACHE = {}

bf16 = ml_dtypes.bfloat16


def _build():
    """Build + compile the per-core Bacc graph (BPC images per core)."""
    import concourse.bacc as bacc
    import concourse.mybir as mybir
    import concourse.tile as tile
    from concourse.bass import ts, ds

    f32 = mybir.dt.float32
    b16 = mybir.dt.bfloat16
    i16 = mybir.dt.int16
    AF = mybir.ActivationFunctionType
    OP = mybir.AluOpType

    nc = bacc.Bacc("TRN2", target_bir_lowering=False, num_devices=NCORES)

    # ---------------- DRAM parameters (host-preprocessed layouts) ----------
    x_ext = nc.declare_dram_parameter("x", [BPC, 2, 128, S], f32, isOutput=False)
    silu_in = nc.declare_dram_parameter("silu_in", [128, 5, BPC], f32, isOutput=False)
    proj_wt = nc.declare_dram_parameter("proj_wt", [128, 5, 512], b16, isOutput=False)
    proj_b = nc.declare_dram_parameter("proj_b", [128, 4, 1], f32, isOutput=False)
    qkw_t = nc.declare_dram_parameter("qkw_t", [128, 2, 512], b16, isOutput=False)
    qk_b = nc.declare_dram_parameter("qk_b", [128, 4, 1], f32, isOutput=False)
    vw_t = nc.declare_dram_parameter("vw_t", [128, 2, 256], b16, isOutput=False)
    outw_t = nc.declare_dram_parameter("outw_t", [128, 2, 256], b16, isOutput=False)
    out_b = nc.declare_dram_parameter("out_b", [1, 256], b16, isOutput=False)
    gnw_p = nc.declare_dram_parameter("gnw", [128, 2, 1], f32, isOutput=False)
    gnb_p = nc.declare_dram_parameter("gnb", [128, 2, 1], f32, isOutput=False)
    ind_g = nc.declare_dram_parameter("ind_g", [128, 2, 8], f32, isOutput=False)
    ind_t = nc.declare_dram_parameter("ind_t", [8, 2, 128], f32, isOutput=False)
    ones32 = nc.declare_dram_parameter("ones32", [128, 32], b16, isOutput=False)
    ones512 = nc.declare_dram_parameter("ones512", [1, 512], b16, isOutput=False)
    out_ext = nc.declare_dram_parameter("out", [BPC, 2, 128, S], f32, isOutput=True)

    exp_counter = [0]

    def exp_engine():
        e = EXP_PATTERN[exp_counter[0] % len(EXP_PATTERN)]
        exp_counter[0] += 1
        return e

    def emit_exp(dst, src):
        e = exp_engine()
        if e == "S":
            nc.scalar.activation(dst, src, AF.Exp)
        else:
            nc.vector.tensor_scalar(
                dst.bitcast(i16), src, A_EXP, B_EXP, OP.mult, OP.add
            )

    with tile.TileContext(nc) as tc:
        with (
            tc.tile_pool(name="const", bufs=1) as const,
            tc.tile_pool(name="xpool", bufs=2 * BPC) as xpool,
            tc.tile_pool(name="xn", bufs=2 * BPC) as xnpool,
            tc.tile_pool(name="qk", bufs=4 * BPC) as qkpool,
            tc.tile_pool(name="vp", bufs=8 * BPC) as vpool,
            tc.tile_pool(name="pp", bufs=8) as ppool,
            tc.tile_pool(name="on", bufs=2 * BPC) as onpool,
            tc.tile_pool(name="sm", bufs=4) as sm,
            tc.tile_pool(name="rd", bufs=2) as rdpool,
            tc.tile_pool(name="yp", bufs=2) as ypool,
            tc.tile_pool(name="psb", bufs=4, space="PSUM") as psb,
        ):
            # ------------- constant / weight loads -------------------------
            projw_sb = const.tile([128, 5, 512], b16)
            nc.sync.dma_start(projw_sb[:], proj_wt[:])
            qkw_sb = const.tile([128, 2, 512], b16)
            nc.sync.dma_start(qkw_sb[:], qkw_t[:])
            vw_sb = const.tile([128, 2, 256], b16)
            nc.sync.dma_start(vw_sb[:], vw_t[:])
            outw_sb = const.tile([128, 2, 256], b16)
            nc.sync.dma_start(outw_sb[:], outw_t[:])
            projb_sb = const.tile([128, 4, 1], f32)
            nc.sync.dma_start(projb_sb[:], proj_b[:])
            qkb_sb = const.tile([128, 4, 1], f32)
            nc.sync.dma_start(qkb_sb[:], qk_b[:])
            outb_sb = const.tile([1, 256], b16)
            nc.sync.dma_start(outb_sb[:], out_b[:])
            gnw_sb = const.tile([128, 2, 1], f32)
            nc.sync.dma_start(gnw_sb[:], gnw_p[:])
            gnb_sb = const.tile([128, 2, 1], f32)
            nc.sync.dma_start(gnb_sb[:], gnb_p[:])
            indg_sb = const.tile([128, 2, 8], f32)
            nc.sync.dma_start(indg_sb[:], ind_g[:])
            indt_sb = const.tile([8, 2, 128], f32)
            nc.sync.dma_start(indt_sb[:], ind_t[:])
            ones32_sb = const.tile([128, 32], b16)
            nc.sync.dma_start(ones32_sb[:], ones32[:])
            ones512_sb = const.tile([1, 512], b16)
            nc.sync.dma_start(ones512_sb[:], ones512[:])
            zrow_sb = const.tile([1, 128], b16)
            nc.vector.memset(zrow_sb[:], 0.0)
            silu_sb = const.tile([128, 5, BPC], f32)
            nc.sync.dma_start(silu_sb[:], silu_in[:])
            eps_sb = const.tile([8, 1], f32)
            nc.vector.memset(eps_sb[:], EPS)

            # ------------- adaLN: silu + projection (both images) ----------
            sige = sm.tile([128, 5, BPC], f32, tag="sm")
            nc.scalar.activation(sige[:], silu_sb[:], AF.Exp, scale=-1.0)
            nc.vector.tensor_scalar_add(sige[:], sige[:], 1.0)
            nc.vector.reciprocal(sige[:], sige[:])
            silu_bf = sm.tile([128, 5, BPC], b16, tag="sm2")
            nc.vector.tensor_tensor(silu_bf[:], silu_sb[:], sige[:], OP.mult)

            params_ps = psb.tile([128, 4 * BPC], f32, tag="ps_b")
            for mt in range(4):
                for kt in range(5):
                    nc.tensor.matmul(
                        params_ps[:, mt * BPC:(mt + 1) * BPC],
                        lhsT=projw_sb[:, kt, ts(mt, 128)],
                        rhs=silu_bf[:, kt, :],
                        start=(kt == 0),
                        stop=(kt == 4),
                    )
            params_sb = sm.tile([128, 4, BPC], f32, tag="sm3")
            for mt in range(4):
                nc.vector.tensor_scalar_add(
                    params_sb[:, mt, :],
                    params_ps[:, mt * BPC:(mt + 1) * BPC],
                    projb_sb[:, mt, :],
                )

            def emit_gn_qkv(b):
                """GroupNorm + adaLN modulation + qkv projections for one
                image. Returns (x_sb, qk_sb, v_sb)."""
                x_sb = []
                for ct in range(2):
                    xt = xpool.tile([128, S], f32, tag="x")
                    nc.sync.dma_start(xt[:], x_ext[b, ct])
                    x_sb.append(xt)

                me2 = []
                for ct in range(2):
                    st6 = sm.tile([128, 2, 6], f32, tag="st6")
                    for half in range(2):
                        nc.vector.bn_stats(
                            st6[:, half, :], x_sb[ct][:, ts(half, 512)]
                        )
                    mv = sm.tile([128, 2], f32, tag="mv")
                    nc.vector.bn_aggr(mv[:], st6[:])
                    m2 = sm.tile([128, 2], f32, tag="m2")
                    nc.vector.tensor_tensor(
                        m2[:, 1:2], mv[:, 0:1], mv[:, 0:1], OP.mult
                    )
                    nc.vector.tensor_tensor(
                        m2[:, 1:2], m2[:, 1:2], mv[:, 1:2], OP.add
                    )
                    nc.vector.tensor_copy(m2[:, 0:1], mv[:, 0:1])
                    me2.append(m2)

                gst_ps = psb.tile([8, 2], f32, tag="ps_b")
                for ct in range(2):
                    nc.tensor.matmul(
                        gst_ps[:],
                        lhsT=indg_sb[:, ct, :],
                        rhs=me2[ct][:],
                        start=(ct == 0),
                        stop=(ct == 1),
                    )
                gst = sm.tile([8, 2], f32, tag="gst")
                nc.vector.tensor_copy(gst[:], gst_ps[:])
                gvar = sm.tile([8, 1], f32, tag="gvar")
                nc.vector.tensor_tensor(gvar[:], gst[:, 0:1], gst[:, 0:1], OP.mult)
                nc.vector.tensor_tensor(gvar[:], gst[:, 1:2], gvar[:], OP.subtract)
                # rstd = exp(-0.5*ln(var+eps)) — keeps everything in the
                # exp/ln activation-table set (no table switch)
                lnv = sm.tile([8, 1], f32, tag="lnv")
                nc.scalar.activation(lnv[:], gvar[:], AF.Ln, bias=eps_sb[:])
                mr = sm.tile([8, 2], f32, tag="mr")
                nc.scalar.activation(mr[:, 1:2], lnv[:], AF.Exp, scale=-0.5)
                nc.vector.tensor_copy(mr[:, 0:1], gst[:, 0:1])

                xn3 = []
                for ct in range(2):
                    pc_ps = psb.tile([128, 2], f32, tag="ps_b")
                    nc.tensor.matmul(
                        pc_ps[:], lhsT=indt_sb[:, ct, :], rhs=mr[:],
                        start=True, stop=True,
                    )
                    gp1 = sm.tile([128, 1], f32, tag="gp1")
                    nc.vector.tensor_scalar_add(
                        gp1[:], params_sb[:, ct, b:b + 1], 1.0
                    )
                    av = sm.tile([128, 1], f32, tag="av")
                    nc.vector.tensor_tensor(av[:], gnw_sb[:, ct, :], gp1[:], OP.mult)
                    nc.vector.tensor_tensor(av[:], pc_ps[:, 1:2], av[:], OP.mult)
                    bv = sm.tile([128, 1], f32, tag="bv")
                    nc.vector.tensor_tensor(bv[:], pc_ps[:, 0:1], av[:], OP.mult)
                    bv2 = sm.tile([128, 1], f32, tag="bv2")
                    nc.vector.tensor_tensor(
                        bv2[:], gnb_sb[:, ct, :], gp1[:], OP.mult
                    )
                    nc.vector.tensor_tensor(bv2[:], bv2[:], bv[:], OP.subtract)
                    nc.vector.tensor_tensor(
                        bv2[:], bv2[:], params_sb[:, 2 + ct, b:b + 1], OP.add
                    )
                    xt = xnpool.tile([128, S], b16, tag="xn")
                    nc.gpsimd.tensor_scalar(
                        xt[:], x_sb[ct][:], av[:], bv2[:], OP.mult, OP.add
                    )
                    xn3.append(xt)

                # ------------- qkv projections -----------------------------
                qk_sb = []
                for mt in range(4):      # 0,1 = q tiles; 2,3 = k tiles
                    ps_qk = psb.tile([128, S], f32, tag="ps_b")
                    for sc in range(2):
                        for kt in range(2):
                            nc.tensor.matmul(
                                ps_qk[:, ts(sc, 512)],
                                lhsT=qkw_sb[:, kt, ts(mt, 128)],
                                rhs=xn3[kt][:, ts(sc, 512)],
                                start=(kt == 0),
                                stop=(kt == 1),
                            )
                    qs = qkpool.tile([128, S], b16, tag="qk")
                    nc.scalar.activation(
                        qs[:], ps_qk[:], AF.Identity, bias=qkb_sb[:, mt, :]
                    )
                    qk_sb.append(qs)

                # v tiles: plain [128 s-strip, 256 vdims] (bias folded into
                # out_b on host)
                v_sb = []
                for st in range(8):
                    ps_v = psb.tile([128, 256], f32, tag="ps_b")
                    for kt in range(2):
                        nc.tensor.matmul(
                            ps_v[:],
                            lhsT=xn3[kt][:, ts(st, 128)],
                            rhs=vw_sb[:, kt, :],
                            start=(kt == 0),
                            stop=(kt == 1),
                        )
                    vt = vpool.tile([128, 256], b16, tag="v")
                    nc.scalar.activation(vt[:], ps_v[:], AF.Copy)
                    v_sb.append(vt)
                return x_sb, qk_sb, v_sb

            def make_attn_stepper(p_sb, v_sb, g4):
                """Returns step(j) emitting the j-th (j=0..15) chunk of the
                attn@v + denominator + normalize work for a finished score
                block: per j one st-step of the col-tiled M=32x4 u chains and
                ones-denominator chains (sc half = j//8, st = j%8). The
                finished [128, S] output tile is returned by step(15)."""
                ot = onpool.tile([128, S], b16, tag="on")
                state = {}

                def step(j):
                    sc, st = divmod(j, 8)
                    if st == 0:
                        ud = psb.tile(
                            [128, 2, 512], f32, tag="ps_b", name="ud"
                        )
                        state["ud"] = ud
                        for slot in range(2):
                            # open the bank's single accumulation group
                            # across all 128 partitions x full 2KB zero
                            # region: K=1 zero-weight matmul writing zeros
                            nc.tensor.matmul(
                                ud[:, slot, :],
                                lhsT=zrow_sb[:],
                                rhs=ones512_sb[:],
                                start=True,
                                stop=False,
                            )
                    ud = state["ud"]
                    for slot in range(2):
                        for hl in range(4):
                            nc.tensor.matmul(
                                ud[ds(32 * hl, 32), slot, :],
                                lhsT=(
                                    v_sb[st][:, ds(32 * (4 * g4 + hl), 32)]
                                    if slot == 0
                                    else ones32_sb[:]
                                ),
                                rhs=p_sb[hl][:, st, ts(sc, 512)],
                                tile_position=(0, 32 * hl),
                                start=False,
                                stop=False,
                            )
                        if st == 7:
                            # close the group across all 128 partitions
                            # (K=1,N=1 zero accumulate, ~60-cycle floor)
                            nc.tensor.matmul(
                                ud[:, slot, 0:1],
                                lhsT=zrow_sb[:],
                                rhs=ones512_sb[:, 0:1],
                                start=False,
                                stop=True,
                            )
                    if st == 7:
                        rden = rdpool.tile([128, 512], f32, tag="rden")
                        nc.vector.reciprocal(rden[:], ud[:, 1, :])
                        nc.vector.tensor_tensor(
                            ot[:, ts(sc, 512)], ud[:, 0, :], rden[:],
                            OP.mult,
                        )
                        # ot partitions 32*hl hold head (4*g4+hl) dims

                return step, ot

            def emit_block(qk_sb, g4, attn_step):
                """Scores (K^T Q, transposed layout) + 3-engine exp for one
                4-head group, interleaved at (pair, st) granularity with the
                previous block's attn steps so the in-order PE queue always
                has matmuls while exps drain. Returns the 4 P tiles."""
                p_sb = [
                    ppool.tile([128, 8, S], b16, tag="p", name=f"p{i}")
                    for i in range(4)
                ]
                j = 0
                for pair in range(2):          # local heads (2*pair, 2*pair+1)
                    for st in range(8):
                        ps_sc = [
                            psb.tile([128, S], f32, tag="ps_b", name=f"sc{i}")
                            for i in range(2)
                        ]
                        for i in range(2):
                            hl = 2 * pair + i          # local head 0..3
                            h = 4 * g4 + hl
                            r = hl * 32                # PE row strip
                            for sc in range(2):
                                nc.tensor.matmul(
                                    ps_sc[i][:, ts(sc, 512)],
                                    lhsT=qk_sb[2 + h // 4][
                                        ds(r, 32), ts(st, 128)
                                    ],
                                    rhs=qk_sb[h // 4][ds(r, 32), ts(sc, 512)],
                                    tile_position=(r, 0),
                                    start=True,
                                    stop=True,
                                )
                        for i in range(2):
                            hl = 2 * pair + i
                            emit_exp(p_sb[hl][:, st, :], ps_sc[i][:])
                        if attn_step is not None:
                            attn_step(j)
                        j += 1
                return p_sb

            def emit_out(b, outn, x_sb):
                """Output projection + bias (ones matmul) + residual."""
                for ct in range(2):
                    for sc in range(2):
                        ps_y = psb.tile([128, 512], f32, tag="ps_b")
                        for o in range(2):
                            nc.tensor.matmul(
                                ps_y[:],
                                lhsT=outw_sb[:, o, ts(ct, 128)],
                                rhs=outn[o][:, ts(sc, 512)],
                                start=(o == 0),
                                stop=False,
                            )
                        nc.tensor.matmul(
                            ps_y[:],
                            lhsT=outb_sb[:, ts(ct, 128)],
                            rhs=ones512_sb[:],
                            start=False,
                            stop=True,
                        )
                        yt = ypool.tile([128, 512], f32, tag="y")
                        nc.vector.tensor_tensor(
                            yt[:], ps_y[:], x_sb[ct][:, ts(sc, 512)],
                            OP.add,
                        )
                        nc.sync.dma_start(
                            out_ext[b, ct, :, ts(sc, 512)], yt[:]
                        )

            # -------- software pipeline over 4 attention blocks ------------
            # block k's score matmuls interleave (per (pair, st) iteration)
            # with block k-1's attn chain steps, so the in-order PE queue
            # never stalls behind exp latency; gn/qkv of image 1 and the
            # out-projections slot into the stream where their inputs close.
            img = [emit_gn_qkv(0)]
            p_blk = [None] * 4
            ot_blk = [None] * 4
            stepper, pending_ot = None, None
            for k in range(2 * BPC):
                b, g4 = divmod(k, 2)
                p_blk[k] = emit_block(img[b][1], g4, stepper)
                if k >= 1:
                    ot_blk[k - 1] = pending_ot
                stepper, pending_ot = make_attn_stepper(
                    p_blk[k], img[b][2], g4
                )
                if k == 0:
                    img.append(emit_gn_qkv(1))
                if k == 2:
                    emit_out(0, [ot_blk[0], ot_blk[1]], img[0][0])
            # drain: attn steps for the last block, bare
            for j in range(16):
                stepper(j)
            ot_blk[3] = pending_ot
            emit_out(1, [ot_blk[2], ot_blk[3]], img[1][0])

    nc.compile()
    return nc


def _prep_consts(inputs):
    """Host-side preprocessing of weights into device layouts (shared by all
    cores). Pure layout/dtype work — the math runs on device."""
    qkv_w = np.asarray(inputs["qkv_w"], np.float32)
    qkv_b = np.asarray(inputs["qkv_b"], np.float32)
    proj_w = np.asarray(inputs["proj_w"], np.float32)
    proj_b = np.asarray(inputs["proj_b"], np.float32)
    out_w = np.asarray(inputs["out_w"], np.float32)
    out_b = np.asarray(inputs["out_b"], np.float32)
    scale = 1.0 / np.sqrt(DK)

    wqk = qkv_w[:512].copy()          # q then k rows
    bqk = qkv_b[:512].copy()
    wqk[:256] *= scale                # fold 1/sqrt(dk) into q
    bqk[:256] *= scale
    wv = qkv_w[512:]
    bv = qkv_b[512:]

    d = {}
    d["proj_wt"] = np.ascontiguousarray(
        proj_w.T.reshape(5, 128, 512).transpose(1, 0, 2)
    ).astype(bf16)
    d["proj_b"] = np.ascontiguousarray(
        proj_b.reshape(4, 128).T.reshape(128, 4, 1)
    )
    d["qkw_t"] = np.ascontiguousarray(
        wqk.T.reshape(2, 128, 512).transpose(1, 0, 2)
    ).astype(bf16)
    d["qk_b"] = np.ascontiguousarray(bqk.reshape(4, 128).T.reshape(128, 4, 1))
    d["vw_t"] = np.ascontiguousarray(
        wv.T.reshape(2, 128, 256).transpose(1, 0, 2)
    ).astype(bf16)
    d["outw_t"] = np.ascontiguousarray(
        out_w.T.reshape(2, 128, 256).transpose(1, 0, 2)
    ).astype(bf16)
    # v bias folded into the out-projection bias: y = (attn+bv)@W^T + b
    #                                               = attn@W^T + (W@bv + b)
    d["out_b"] = (out_b + out_w @ bv).reshape(1, 256).astype(bf16)
    d["gnw"] = np.ascontiguousarray(
        np.asarray(inputs["gn_weight"], np.float32).reshape(2, 128).T
    ).reshape(128, 2, 1)
    d["gnb"] = np.ascontiguousarray(
        np.asarray(inputs["gn_bias"], np.float32).reshape(2, 128).T
    ).reshape(128, 2, 1)

    ind_g = np.zeros((128, 2, 8), np.float32)
    ind_t = np.zeros((8, 2, 128), np.float32)
    for ct in range(2):
        for p in range(128):
            g = (ct * 128 + p) // 32
            ind_g[p, ct, g] = 1.0 / 32.0
            ind_t[g, ct, p] = 1.0
    d["ind_g"] = ind_g
    d["ind_t"] = ind_t
    d["ones32"] = np.ones((128, 32), bf16)
    d["ones512"] = np.ones((1, 512), bf16)
    return d


def make_in_maps(inputs):
    consts = _prep_consts(inputs)
    x = np.asarray(inputs["x"], np.float32).reshape(B, 2, 128, S)
    t_emb = np.asarray(inputs["t_emb"], np.float32)
    cond_emb = np.asarray(inputs["cond_emb"], np.float32)
    inp_all = np.concatenate([t_emb, cond_emb], axis=1)       # (B, 640)

    in_maps = []
    for c in range(NCORES):
        m = dict(consts)
        m["x"] = np.ascontiguousarray(x[c * BPC:(c + 1) * BPC])
        sl = inp_all[c * BPC:(c + 1) * BPC].T                 # (640, BPC)
        m["silu_in"] = np.ascontiguousarray(
            sl.reshape(5, 128, BPC).transpose(1, 0, 2)
        )
        in_maps.append(m)
    return in_maps


def run(inputs, trace=False):
    from concourse.bass_utils import run_bass_kernel_spmd

    if "nc" not in _CACHE:
        _CACHE["nc"] = _build()
    nc = _CACHE["nc"]
    in_maps = make_in_maps(inputs)
    res = run_bass_kernel_spmd(
        nc, in_maps, core_ids=list(range(NCORES)), trace=trace
    )
    outs = [
        res.results[c]["out"].reshape(BPC, 256, HH, WW) for c in range(NCORES)
    ]
    y = np.concatenate(outs, axis=0)
    return y, res.exec_time_ns


def kernel(**inputs):
    y, _ = run(inputs, trace=False)
    return y


# revision 23
# speedup vs baseline: 1.1965x; 1.0250x over previous
"""AttentionBlock (adaptive GroupNorm + spatial self-attention + residual)
Trainium2 Bass/Tile kernel, data-parallel over batch across 8 NeuronCores.

Reference computation (B=16, C=256, H=W=32, 8 heads x 32 dk, 8 GN groups):
  params = silu([t_emb, cond_emb]) @ proj_w.T + proj_b       (B, 512)
  xn = GroupNorm(x) * (1+gamma) + beta                        (B, C, 1024)
  qkv = xn.T @ qkv_w.T + qkv_b ; attention over 1024 positions
  out = attn_out @ out_w.T + out_b ; y = out + x

Per-core layout strategy (2 images/core):
  - everything kept in [channel/partition, spatial/free] layout
  - scores computed transposed  S_T[t, s] = K^T Q  via 4-head row-tiled
    matmuls (K=32 contraction, tile_position row strips)
  - softmax exp is the #1 bottleneck (8 * 1024^2 exps/image); it is split
    7:6 across ScalarE (exact activation exp) and DVE (Schraudolph
    bit-trick exp: bf16 bits = int16(x*128/ln2 + 16250.5) via one
    tensor_scalar into an int16-bitcast view; ~1.7% mean element error
    that largely cancels in softmax normalization). GPSIMD cannot read
    PSUM on HW, so it only handles the SBUF-side adaLN modulation.
  - attn@v as 4-head col-tiled M=32 chains into one PSUM bank; softmax
    denominators via a second 4-head col-tiled ones-matmul pass into the
    sibling bank with IDENTICAL row alignment, so ONE full-width DVE
    reciprocal + ONE tensor_tensor normalizes all 4 heads at once.
    Each bank's single accumulation group is opened by a K=1 N=512
    zero-weight matmul spanning all 128 partitions and closed by a
    K=1 N=1 dummy (PSUM zero-region rule).
  - the 4 attention blocks (2 images x 2 head-groups) are software-
    pipelined: block k's score matmuls interleave per (pair, st) with
    block k-1's attn/denominator chain steps so the in-order PE queue
    always has work while exps drain on ScalarE/DVE
  - v bias folded into out_b on the host (out_b' = out_b + out_w @ v_b)
  - qk biases ride the mandatory PSUM->SBUF copy on ScalarE
    (AF.Identity with per-partition bias, same act table set as exp)
"""

import numpy as np
import ml_dtypes

B, C, HH, WW = 16, 256, 32, 32
S = HH * WW              # 1024
NH, DK = 8, 32           # heads x head_dim
G = 8                    # groupnorm groups
T_DIM, COND_DIM = 512, 128
IN_DIM = T_DIM + COND_DIM
EPS = 1e-6
NCORES = 8
BPC = B // NCORES        # images per core

# Schraudolph exp constants for bf16-bits-via-int16:
#   bits = int16(x * 128/ln2 + (127*128 - 5.5))
A_EXP = 128.0 / np.log(2.0)
B_EXP = 127.0 * 128.0 - 5.5

# exp engine rotation: S=ScalarE exact, D=DVE schraudolph, P=GPSIMD schraudolph
EXP_PATTERN = "SDSDSDSDSDSDS"

_CACHE = {}

bf16 = ml_dtypes.bfloat16


def _build():
    """Build + compile the per-core Bacc graph (BPC images per core)."""
    import concourse.bacc as bacc
    import concourse.mybir as mybir
    import concourse.tile as tile
    from concourse.bass import ts, ds

    f32 = mybir.dt.float32
    b16 = mybir.dt.bfloat16
    i16 = mybir.dt.int16
    AF = mybir.ActivationFunctionType
    OP = mybir.AluOpType

    nc = bacc.Bacc("TRN2", target_bir_lowering=False, num_devices=NCORES)

    # ---------------- DRAM parameters (host-preprocessed layouts) ----------
    x_ext = nc.declare_dram_parameter("x", [BPC, 2, 128, S], f32, isOutput=False)
    silu_in = nc.declare_dram_parameter("silu_in", [128, 5, BPC], f32, isOutput=False)
    proj_wt = nc.declare_dram_parameter("proj_wt", [128, 5, 512], b16, isOutput=False)
    proj_b = nc.declare_dram_parameter("proj_b", [128, 4, 1], f32, isOutput=False)
    qkw_t = nc.declare_dram_parameter("qkw_t", [128, 2, 512], b16, isOutput=False)
    qk_b = nc.declare_dram_parameter("qk_b", [128, 4, 1], f32, isOutput=False)
    vw_t = nc.declare_dram_parameter("vw_t", [128, 2, 256], b16, isOutput=False)
    outw_t = nc.declare_dram_parameter("outw_t", [128, 2, 256], b16, isOutput=False)
    out_b = nc.declare_dram_parameter("out_b", [1, 256], b16, isOutput=False)
    gnw_p = nc.declare_dram_parameter("gnw", [128, 2, 1], f32, isOutput=False)
    gnb_p = nc.declare_dram_parameter("gnb", [128, 2, 1], f32, isOutput=False)
    ind_g = nc.declare_dram_parameter("ind_g", [128, 2, 8], f32, isOutput=False)
    ind_t = nc.declare_dram_parameter("ind_t", [8, 2, 128], f32, isOutput=False)
    ones32 = nc.declare_dram_parameter("ones32", [128, 32], b16, isOutput=False)
    ones512 = nc.declare_dram_parameter("ones512", [1, 512], b16, isOutput=False)
    out_ext = nc.declare_dram_parameter("out", [BPC, 2, 128, S], f32, isOutput=True)

    exp_counter = [0]

    def exp_engine():
        e = EXP_PATTERN[exp_counter[0] % len(EXP_PATTERN)]
        exp_counter[0] += 1
        return e

    def emit_exp(dst, src):
        e = exp_engine()
        if e == "S":
            nc.scalar.activation(dst, src, AF.Exp)
        else:
            nc.vector.tensor_scalar(
                dst.bitcast(i16), src, A_EXP, B_EXP, OP.mult, OP.add
            )

    with tile.TileContext(nc) as tc:
        with (
            tc.tile_pool(name="const", bufs=1) as const,
            tc.tile_pool(name="xpool", bufs=2 * BPC) as xpool,
            tc.tile_pool(name="xn", bufs=2 * BPC) as xnpool,
            tc.tile_pool(name="qk", bufs=4 * BPC) as qkpool,
            tc.tile_pool(name="vp", bufs=8 * BPC) as vpool,
            tc.tile_pool(name="pp", bufs=8) as ppool,
            tc.tile_pool(name="on", bufs=3) as onpool,
            tc.tile_pool(name="sm", bufs=4) as sm,
            tc.tile_pool(name="rd", bufs=2) as rdpool,
            tc.tile_pool(name="yp", bufs=2) as ypool,
            tc.tile_pool(name="psb", bufs=4, space="PSUM") as psb,
        ):
            # ------------- constant / weight loads -------------------------
            projw_sb = const.tile([128, 5, 512], b16)
            nc.sync.dma_start(projw_sb[:], proj_wt[:])
            qkw_sb = const.tile([128, 2, 512], b16)
            nc.sync.dma_start(qkw_sb[:], qkw_t[:])
            vw_sb = const.tile([128, 2, 256], b16)
            nc.sync.dma_start(vw_sb[:], vw_t[:])
            outw_sb = const.tile([128, 2, 256], b16)
            nc.sync.dma_start(outw_sb[:], outw_t[:])
            projb_sb = const.tile([128, 4, 1], f32)
            nc.sync.dma_start(projb_sb[:], proj_b[:])
            qkb_sb = const.tile([128, 4, 1], f32)
            nc.sync.dma_start(qkb_sb[:], qk_b[:])
            outb_sb = const.tile([1, 256], b16)
            nc.sync.dma_start(outb_sb[:], out_b[:])
            gnw_sb = const.tile([128, 2, 1], f32)
            nc.sync.dma_start(gnw_sb[:], gnw_p[:])
            gnb_sb = const.tile([128, 2, 1], f32)
            nc.sync.dma_start(gnb_sb[:], gnb_p[:])
            indg_sb = const.tile([128, 2, 8], f32)
            nc.sync.dma_start(indg_sb[:], ind_g[:])
            indt_sb = const.tile([8, 2, 128], f32)
            nc.sync.dma_start(indt_sb[:], ind_t[:])
            ones32_sb = const.tile([128, 32], b16)
            nc.sync.dma_start(ones32_sb[:], ones32[:])
            ones512_sb = const.tile([1, 512], b16)
            nc.sync.dma_start(ones512_sb[:], ones512[:])
            zrow_sb = const.tile([1, 128], b16)
            nc.vector.memset(zrow_sb[:], 0.0)
            silu_sb = const.tile([128, 5, BPC], f32)
            nc.sync.dma_start(silu_sb[:], silu_in[:])
            eps_sb = const.tile([8, 1], f32)
            nc.vector.memset(eps_sb[:], EPS)

            # ------------- adaLN: silu + projection (both images) ----------
            sige = sm.tile([128, 5, BPC], f32, tag="sm")
            nc.scalar.activation(sige[:], silu_sb[:], AF.Exp, scale=-1.0)
            nc.vector.tensor_scalar_add(sige[:], sige[:], 1.0)
            nc.vector.reciprocal(sige[:], sige[:])
            silu_bf = sm.tile([128, 5, BPC], b16, tag="sm2")
            nc.vector.tensor_tensor(silu_bf[:], silu_sb[:], sige[:], OP.mult)

            params_ps = psb.tile([128, 4 * BPC], f32, tag="ps_b")
            for mt in range(4):
                for kt in range(5):
                    nc.tensor.matmul(
                        params_ps[:, mt * BPC:(mt + 1) * BPC],
                        lhsT=projw_sb[:, kt, ts(mt, 128)],
                        rhs=silu_bf[:, kt, :],
                        start=(kt == 0),
                        stop=(kt == 4),
                    )
            params_sb = sm.tile([128, 4, BPC], f32, tag="sm3")
            for mt in range(4):
                nc.vector.tensor_scalar_add(
                    params_sb[:, mt, :],
                    params_ps[:, mt * BPC:(mt + 1) * BPC],
                    projb_sb[:, mt, :],
                )

            def emit_gn_qkv(b):
                """GroupNorm + adaLN modulation + qkv projections for one
                image. Returns (x_sb, qk_sb, v_sb)."""
                x_sb = []
                for ct in range(2):
                    xt = xpool.tile([128, S], f32, tag="x")
                    nc.sync.dma_start(xt[:], x_ext[b, ct])
                    x_sb.append(xt)

                me2 = []
                for ct in range(2):
                    st6 = sm.tile([128, 2, 6], f32, tag="st6")
                    for half in range(2):
                        nc.vector.bn_stats(
                            st6[:, half, :], x_sb[ct][:, ts(half, 512)]
                        )
                    mv = sm.tile([128, 2], f32, tag="mv")
                    nc.vector.bn_aggr(mv[:], st6[:])
                    m2 = sm.tile([128, 2], f32, tag="m2")
                    nc.vector.tensor_tensor(
                        m2[:, 1:2], mv[:, 0:1], mv[:, 0:1], OP.mult
                    )
                    nc.vector.tensor_tensor(
                        m2[:, 1:2], m2[:, 1:2], mv[:, 1:2], OP.add
                    )
                    nc.vector.tensor_copy(m2[:, 0:1], mv[:, 0:1])
                    me2.append(m2)

                gst_ps = psb.tile([8, 2], f32, tag="ps_b")
                for ct in range(2):
                    nc.tensor.matmul(
                        gst_ps[:],
                        lhsT=indg_sb[:, ct, :],
                        rhs=me2[ct][:],
                        start=(ct == 0),
                        stop=(ct == 1),
                    )
                gst = sm.tile([8, 2], f32, tag="gst")
                nc.vector.tensor_copy(gst[:], gst_ps[:])
                gvar = sm.tile([8, 1], f32, tag="gvar")
                nc.vector.tensor_tensor(gvar[:], gst[:, 0:1], gst[:, 0:1], OP.mult)
                nc.vector.tensor_tensor(gvar[:], gst[:, 1:2], gvar[:], OP.subtract)
                # rstd = exp(-0.5*ln(var+eps)) — keeps everything in the
                # exp/ln activation-table set (no table switch)
                lnv = sm.tile([8, 1], f32, tag="lnv")
                nc.scalar.activation(lnv[:], gvar[:], AF.Ln, bias=eps_sb[:])
                mr = sm.tile([8, 2], f32, tag="mr")
                nc.scalar.activation(mr[:, 1:2], lnv[:], AF.Exp, scale=-0.5)
                nc.vector.tensor_copy(mr[:, 0:1], gst[:, 0:1])

                xn3 = []
                for ct in range(2):
                    pc_ps = psb.tile([128, 2], f32, tag="ps_b")
                    nc.tensor.matmul(
                        pc_ps[:], lhsT=indt_sb[:, ct, :], rhs=mr[:],
                        start=True, stop=True,
                    )
                    gp1 = sm.tile([128, 1], f32, tag="gp1")
                    nc.vector.tensor_scalar_add(
                        gp1[:], params_sb[:, ct, b:b + 1], 1.0
                    )
                    av = sm.tile([128, 1], f32, tag="av")
                    nc.vector.tensor_tensor(av[:], gnw_sb[:, ct, :], gp1[:], OP.mult)
                    nc.vector.tensor_tensor(av[:], pc_ps[:, 1:2], av[:], OP.mult)
                    bv = sm.tile([128, 1], f32, tag="bv")
                    nc.vector.tensor_tensor(bv[:], pc_ps[:, 0:1], av[:], OP.mult)
                    bv2 = sm.tile([128, 1], f32, tag="bv2")
                    nc.vector.tensor_tensor(
                        bv2[:], gnb_sb[:, ct, :], gp1[:], OP.mult
                    )
                    nc.vector.tensor_tensor(bv2[:], bv2[:], bv[:], OP.subtract)
                    nc.vector.tensor_tensor(
                        bv2[:], bv2[:], params_sb[:, 2 + ct, b:b + 1], OP.add
                    )
                    xt = xnpool.tile([128, S], b16, tag="xn")
                    nc.gpsimd.tensor_scalar(
                        xt[:], x_sb[ct][:], av[:], bv2[:], OP.mult, OP.add
                    )
                    xn3.append(xt)

                # ------------- qkv projections -----------------------------
                qk_sb = []
                for mt in range(4):      # 0,1 = q tiles; 2,3 = k tiles
                    ps_qk = psb.tile([128, S], f32, tag="ps_b")
                    for sc in range(2):
                        for kt in range(2):
                            nc.tensor.matmul(
                                ps_qk[:, ts(sc, 512)],
                                lhsT=qkw_sb[:, kt, ts(mt, 128)],
                                rhs=xn3[kt][:, ts(sc, 512)],
                                start=(kt == 0),
                                stop=(kt == 1),
                            )
                    qs = qkpool.tile([128, S], b16, tag="qk")
                    nc.scalar.activation(
                        qs[:], ps_qk[:], AF.Identity, bias=qkb_sb[:, mt, :]
                    )
                    qk_sb.append(qs)

                # v tiles: plain [128 s-strip, 256 vdims] (bias folded into
                # out_b on host)
                v_sb = []
                for st in range(8):
                    ps_v = psb.tile([128, 256], f32, tag="ps_b")
                    for kt in range(2):
                        nc.tensor.matmul(
                            ps_v[:],
                            lhsT=xn3[kt][:, ts(st, 128)],
                            rhs=vw_sb[:, kt, :],
                            start=(kt == 0),
                            stop=(kt == 1),
                        )
                    vt = vpool.tile([128, 256], b16, tag="v")
                    nc.scalar.activation(vt[:], ps_v[:], AF.Copy)
                    v_sb.append(vt)
                return x_sb, qk_sb, v_sb

            def make_attn_stepper(p_sb, v_sb, g4):
                """Returns step(j) emitting the j-th (j=0..15) chunk of the
                attn@v + denominator + normalize work for a finished score
                block: per j one st-step of the col-tiled M=32x4 u chains and
                ones-denominator chains (sc half = j//8, st = j%8). The
                finished [128, S] output tile is returned by step(15)."""
                ot = onpool.tile([128, S], b16, tag="on")
                state = {}

                def step(j):
                    sc, st = divmod(j, 8)
                    if st == 0:
                        ud = psb.tile(
                            [128, 2, 512], f32, tag="ps_b", name="ud"
                        )
                        state["ud"] = ud
                        for slot in range(2):
                            # open the bank's single accumulation group
                            # across all 128 partitions x full 2KB zero
                            # region: K=1 zero-weight matmul writing zeros
                            nc.tensor.matmul(
                                ud[:, slot, :],
                                lhsT=zrow_sb[:],
                                rhs=ones512_sb[:],
                                start=True,
                                stop=False,
                            )
                    ud = state["ud"]
                    for slot in range(2):
                        for hl in range(4):
                            nc.tensor.matmul(
                                ud[ds(32 * hl, 32), slot, :],
                                lhsT=(
                                    v_sb[st][:, ds(32 * (4 * g4 + hl), 32)]
                                    if slot == 0
                                    else ones32_sb[:]
                                ),
                                rhs=p_sb[hl][:, st, ts(sc, 512)],
                                tile_position=(0, 32 * hl),
                                start=False,
                                stop=False,
                            )
                        if st == 7:
                            # close the group across all 128 partitions
                            # (K=1,N=1 zero accumulate, ~60-cycle floor)
                            nc.tensor.matmul(
                                ud[:, slot, 0:1],
                                lhsT=zrow_sb[:],
                                rhs=ones512_sb[:, 0:1],
                                start=False,
                                stop=True,
                            )
                    if st == 7:
                        rden = rdpool.tile([128, 512], f32, tag="rden")
                        nc.vector.reciprocal(rden[:], ud[:, 1, :])
                        nc.vector.tensor_tensor(
                            ot[:, ts(sc, 512)], ud[:, 0, :], rden[:],
                            OP.mult,
                        )
                        # ot partitions 32*hl hold head (4*g4+hl) dims

                return step, ot

            def emit_block(qk_sb, g4, attn_step):
                """Scores (K^T Q, transposed layout) + 3-engine exp for one
                4-head group, interleaved at (pair, st) granularity with the
                previous block's attn steps so the in-order PE queue always
                has matmuls while exps drain. Returns the 4 P tiles."""
                p_sb = [
                    ppool.tile([128, 8, S], b16, tag="p", name=f"p{i}")
                    for i in range(4)
                ]
                j = 0
                for pair in range(2):          # local heads (2*pair, 2*pair+1)
                    for st in range(8):
                        ps_sc = [
                            psb.tile([128, S], f32, tag="ps_b", name=f"sc{i}")
                            for i in range(2)
                        ]
                        for i in range(2):
                            hl = 2 * pair + i          # local head 0..3
                            h = 4 * g4 + hl
                            r = hl * 32                # PE row strip
                            for sc in range(2):
                                nc.tensor.matmul(
                                    ps_sc[i][:, ts(sc, 512)],
                                    lhsT=qk_sb[2 + h // 4][
                                        ds(r, 32), ts(st, 128)
                                    ],
                                    rhs=qk_sb[h // 4][ds(r, 32), ts(sc, 512)],
                                    tile_position=(r, 0),
                                    start=True,
                                    stop=True,
                                )
                        for i in range(2):
                            hl = 2 * pair + i
                            emit_exp(p_sb[hl][:, st, :], ps_sc[i][:])
                        if attn_step is not None:
                            attn_step(j)
                        j += 1
                return p_sb

            def emit_out(b, outn, x_sb):
                """Output projection + bias (ones matmul) + residual."""
                for ct in range(2):
                    for sc in range(2):
                        ps_y = psb.tile([128, 512], f32, tag="ps_b")
                        for o in range(2):
                            nc.tensor.matmul(
                                ps_y[:],
                                lhsT=outw_sb[:, o, ts(ct, 128)],
                                rhs=outn[o][:, ts(sc, 512)],
                                start=(o == 0),
                                stop=False,
                            )
                        nc.tensor.matmul(
                            ps_y[:],
                            lhsT=outb_sb[:, ts(ct, 128)],
                            rhs=ones512_sb[:],
                            start=False,
                            stop=True,
                        )
                        yt = ypool.tile([128, 512], f32, tag="y")
                        nc.vector.tensor_tensor(
                            yt[:], ps_y[:], x_sb[ct][:, ts(sc, 512)],
                            OP.add,
                        )
                        nc.sync.dma_start(
                            out_ext[b, ct, :, ts(sc, 512)], yt[:]
                        )

            # -------- software pipeline over 4 attention blocks ------------
            # block k's score matmuls interleave (per (pair, st) iteration)
            # with block k-1's attn chain steps, so the in-order PE queue
            # never stalls behind exp latency; gn/qkv of image 1 and the
            # out-projections slot into the stream where their inputs close.
            img = [emit_gn_qkv(0)]
            p_blk = [None] * 4
            ot_blk = [None] * 4
            stepper, pending_ot = None, None
            for k in range(2 * BPC):
                b, g4 = divmod(k, 2)
                p_blk[k] = emit_block(img[b][1], g4, stepper)
                if k >= 1:
                    ot_blk[k - 1] = pending_ot
                stepper, pending_ot = make_attn_stepper(
                    p_blk[k], img[b][2], g4
                )
                if k == 0:
                    img.append(emit_gn_qkv(1))
                if k == 2:
                    emit_out(0, [ot_blk[0], ot_blk[1]], img[0][0])
            # drain: attn steps for the last block, bare
            for j in range(16):
                stepper(j)
            ot_blk[3] = pending_ot
            emit_out(1, [ot_blk[2], ot_blk[3]], img[1][0])

    nc.compile()
    return nc


def _prep_consts(inputs):
    """Host-side preprocessing of weights into device layouts (shared by all
    cores). Pure layout/dtype work — the math runs on device."""
    qkv_w = np.asarray(inputs["qkv_w"], np.float32)
    qkv_b = np.asarray(inputs["qkv_b"], np.float32)
    proj_w = np.asarray(inputs["proj_w"], np.float32)
    proj_b = np.asarray(inputs["proj_b"], np.float32)
    out_w = np.asarray(inputs["out_w"], np.float32)
    out_b = np.asarray(inputs["out_b"], np.float32)
    scale = 1.0 / np.sqrt(DK)

    wqk = qkv_w[:512].copy()          # q then k rows
    bqk = qkv_b[:512].copy()
    wqk[:256] *= scale                # fold 1/sqrt(dk) into q
    bqk[:256] *= scale
    wv = qkv_w[512:]
    bv = qkv_b[512:]

    d = {}
    d["proj_wt"] = np.ascontiguousarray(
        proj_w.T.reshape(5, 128, 512).transpose(1, 0, 2)
    ).astype(bf16)
    d["proj_b"] = np.ascontiguousarray(
        proj_b.reshape(4, 128).T.reshape(128, 4, 1)
    )
    d["qkw_t"] = np.ascontiguousarray(
        wqk.T.reshape(2, 128, 512).transpose(1, 0, 2)
    ).astype(bf16)
    d["qk_b"] = np.ascontiguousarray(bqk.reshape(4, 128).T.reshape(128, 4, 1))
    d["vw_t"] = np.ascontiguousarray(
        wv.T.reshape(2, 128, 256).transpose(1, 0, 2)
    ).astype(bf16)
    d["outw_t"] = np.ascontiguousarray(
        out_w.T.reshape(2, 128, 256).transpose(1, 0, 2)
    ).astype(bf16)
    # v bias folded into the out-projection bias: y = (attn+bv)@W^T + b
    #                                               = attn@W^T + (W@bv + b)
    d["out_b"] = (out_b + out_w @ bv).reshape(1, 256).astype(bf16)
    d["gnw"] = np.ascontiguousarray(
        np.asarray(inputs["gn_weight"], np.float32).reshape(2, 128).T
    ).reshape(128, 2, 1)
    d["gnb"] = np.ascontiguousarray(
        np.asarray(inputs["gn_bias"], np.float32).reshape(2, 128).T
    ).reshape(128, 2, 1)

    ind_g = np.zeros((128, 2, 8), np.float32)
    ind_t = np.zeros((8, 2, 128), np.float32)
    for ct in range(2):
        for p in range(128):
            g = (ct * 128 + p) // 32
            ind_g[p, ct, g] = 1.0 / 32.0
            ind_t[g, ct, p] = 1.0
    d["ind_g"] = ind_g
    d["ind_t"] = ind_t
    d["ones32"] = np.ones((128, 32), bf16)
    d["ones512"] = np.ones((1, 512), bf16)
    return d


def make_in_maps(inputs):
    consts = _prep_consts(inputs)
    x = np.asarray(inputs["x"], np.float32).reshape(B, 2, 128, S)
    t_emb = np.asarray(inputs["t_emb"], np.float32)
    cond_emb = np.asarray(inputs["cond_emb"], np.float32)
    inp_all = np.concatenate([t_emb, cond_emb], axis=1)       # (B, 640)

    in_maps = []
    for c in range(NCORES):
        m = dict(consts)
        m["x"] = np.ascontiguousarray(x[c * BPC:(c + 1) * BPC])
        sl = inp_all[c * BPC:(c + 1) * BPC].T                 # (640, BPC)
        m["silu_in"] = np.ascontiguousarray(
            sl.reshape(5, 128, BPC).transpose(1, 0, 2)
        )
        in_maps.append(m)
    return in_maps


def run(inputs, trace=False):
    from concourse.bass_utils import run_bass_kernel_spmd

    if "nc" not in _CACHE:
        _CACHE["nc"] = _build()
    nc = _CACHE["nc"]
    in_maps = make_in_maps(inputs)
    res = run_bass_kernel_spmd(
        nc, in_maps, core_ids=list(range(NCORES)), trace=trace
    )
    outs = [
        res.results[c]["out"].reshape(BPC, 256, HH, WW) for c in range(NCORES)
    ]
    y = np.concatenate(outs, axis=0)
    return y, res.exec_time_ns


def kernel(**inputs):
    y, _ = run(inputs, trace=False)
    return y
